# revision 1
# baseline (speedup 1.0000x reference)
"""DropPart masking kernel for Trainium2 (8 NeuronCores, data-parallel over batch).

Problem: x (64, 256, 96, 32) f32. Per sample n and channel-group g (8 groups x
32 channels), a keypoint defines a keep-box; if roll[n,g] < 0.5 the group's
channels are zeroed outside the box (box <= 16x16 in the 96x32 image), else the
group passes through unchanged.

The op is pure data movement plus a 0/1 mask multiply: every (n,g) "slot"
([32ch, 96, 32] = 384KB) is either identity or x*boxmask. This kernel runs
IN-PLACE on the device: the full x tensor is donated as the buffer backing the
NEFF's output (XLA donation aliases the param to the custom-call result -- the
same mechanism bass2jax.run_bass_via_pjrt relies on for its zero-filled output
buffers), so identity slots need ZERO HBM traffic and only masked slots are
touched: one 384KB slab read + one 384KB slab write each. At ~50% masked slots
that is ~32MB/core round-trip vs 50MB for the out-of-place streaming kernel,
and the f32 output is bit-exact (rel err 0.0).

Which slots are masked is per-core data, so a single SPMD program executes a
metadata-driven schedule: per work position an i32 input supplies (slot s,
flag f); the slab load/store DMAs take their DRAM offsets from engine
registers (values_load) and padding positions are skipped via cond=f
(skip_entire_dma). dep_tracking_offset on each dynamic AP is pinned to a
static per-position fake region so the Tile scheduler sees disjoint slots
instead of serializing every DMA against the whole output tensor. The mask
multiply uses the eyes-matmul expansion (PE, [4,128] one-hot weights applied
to 4 packed bf16 mask rows) + DVE tensor_mul; masked-slot masks are 1 inside
the keep-box and 0 outside, so the product equals the reference's x*mask
bit-for-bit, and all-ones lanes pass identity data through unchanged. Slab
DMAs rotate across three queues (SP + ACT HWDGE, gpsimd SWDGE) -- the
measured bottleneck is per-dynamic-DMA queue overhead (~2-3us each), not
bandwidth, so the third queue and the one-load-one-store-per-slot structure
are what set the 129us/iter (vs 154us baseline full-stream) timing.

The program is input-independent (one compile per work-list capacity, cached);
the For_i(nreps) wrapper supports in-NEFF repetition for timing. The body is
idempotent (mask o mask = mask), so repeated in-place application is safe.
"""

import numpy as np
import ml_dtypes

import bass_rust
import concourse.bass as bass
import concourse.bacc as bacc
import concourse.tile as tile
from concourse import mybir

N, C, H, W = 64, 256, 96, 32
GROUPS = 8
P_DROP = 0.5
HW = H * W              # 3072
CHS = C // GROUPS       # 32
N_CORES = 8
NPC = N // N_CORES      # samples per core = 8
SLOTS = NPC * GROUPS    # (sample, group) slots per core = 64
NCHUNK = 6              # 512-element chunks per image (96*32 / 512)

_F32 = mybir.dt.float32
_BF16 = mybir.dt.bfloat16
_I32 = mybir.dt.int32

_SP = mybir.EngineType.SP
_ACT = mybir.EngineType.Activation
_POOL = mybir.EngineType.Pool


SLAB = CHS * HW  # elements per slot slab = 98304


def _win_ap(o4, s, c, p):
    """[32, 2, 512] window AP at slab s, chunks [c, c+2): widen the
    [32, 512] single-chunk slice with an extra (stride=512, size=2) dim.

    dep_tracking_offset is pinned to a static per-work-position fake slab so
    the Tile dependency tracker sees disjoint regions per slot (the dynamic
    offsets would otherwise serialize every DMA against the whole tensor).
    Within a position, the window load / zero store / window store share the
    fake region, preserving their required ordering; distinct positions touch
    provably disjoint real slabs, so dropping those edges is sound."""
    a = o4[s, :, c, :].copy()
    a.ap = bass_rust.VecI64Pair([(HW, CHS), (512, 2), (1, 512)])
    a.dep_tracking_offset = p * SLAB
    return a


def _slab_ap(o4, s, p):
    """[32, 6, 512] full-slab AP at slab s, same fake dep region as _win_ap."""
    a = o4[s].copy()
    a.dep_tracking_offset = p * SLAB
    return a


def _nslab_ap(o4, s, p, npart):
    """[npart*32, 6, 512] AP over npart consecutive slabs starting at slab s.
    Fake dep region: 2 slabs per work position (positions never overlap)."""
    a = o4[s].copy()
    a.ap = bass_rust.VecI64Pair([(HW, npart * CHS), (512, NCHUNK), (1, 512)])
    a.dep_tracking_offset = p * 2 * SLAB
    return a


def _build_module(w_key, xbufs: int = 6, mulw: int = 1024):
    """One SPMD module processing w_items*4 slot positions per iteration.

    Per slot position p (metadata: slot s, active flag f), all offsets coming
    from engine registers so one compiled program serves every core's
    schedule:
      L: DMA slab s ([32, 6, 512] f32, 384KB) into the item's [128, 3072]
         SBUF tile (cond=f; inactive positions skip via skip_entire_dma).
      mask multiply: PE expands the 4 packed mask rows (one per 32-channel
         lane) to 128 partitions via the eyes matmul; DVE multiplies the
         whole tile. Masked-slot masks are 1 inside the keep-box and 0
         outside, so the product is exactly the reference's x*mask.
      S: DMA the slab back (cond=f). S chains after L through the SBUF
         dependency (mul), so queues are free; cross-iteration window races
         in the timing loop are benign because masking is idempotent.
    L and S rotate across the SP / ACT HWDGE queues and the gpsimd SWDGE
    queue to spread descriptor-processing overhead and store bandwidth.
    """
    w2, w1 = w_key
    npos = w2 * 2 + w1 * 4
    w_items = w2 + w1
    nc = bacc.Bacc("TRN2", target_bir_lowering=False, debug=False)

    o4 = nc.dram_tensor("out", [SLOTS, CHS, NCHUNK, 512], _F32, kind="ExternalOutput").ap()
    meta_d = nc.dram_tensor("meta", [1, npos * 4], _I32, kind="ExternalInput").ap()
    mpk_d = nc.dram_tensor("mpk", [4, w_items * HW], _BF16, kind="ExternalInput").ap()
    eyes_d = nc.dram_tensor("eyes", [4, 128], _BF16, kind="ExternalInput").ap()
    reps_d = nc.dram_tensor("nreps", [1, 1], _I32, kind="ExternalInput").ap()

    queues = [(nc.sync, _SP), (nc.scalar, _ACT), (nc.gpsimd, _POOL)]
    nbank = mulw // 512

    with tile.TileContext(nc) as tc:
        with (
            tc.tile_pool(name="consts", bufs=1) as consts,
            tc.tile_pool(name="xpool", bufs=xbufs) as xpool,
            tc.tile_pool(name="psum", bufs=8 // nbank, space="PSUM") as psum,
        ):
            mt = consts.tile([1, npos * 4], _I32)
            nc.sync.dma_start(mt[:], meta_d[:])
            mpk = consts.tile([4, w_items * HW], _BF16)
            nc.sync.dma_start(mpk[:], mpk_d[:])
            eyes = consts.tile([4, 128], _BF16)
            nc.sync.dma_start(eyes[:], eyes_d[:])
            rtile = consts.tile([1, 1], _I32)
            nc.sync.dma_start(rtile[:], reps_d[:])

            dmac = [0]  # global DMA counter for queue rotation

            def unit_io(it, units, npart):
                """Loads for one item: `units` dynamic units of `npart*32`
                partitions each. Returns regs for the store pass."""
                xt = xpool.tile([128, HW], _F32)
                regs = []
                for u in range(units):
                    p = it_pos(it, u, units)
                    qL, engL = queues[dmac[0] % 3]
                    qS, engS = queues[(dmac[0] + 1) % 3]
                    dmac[0] += 1
                    engs = [engL] if engS == engL else [engL, engS]
                    s = nc.values_load(mt[0:1, 4 * p : 4 * p + 1], engines=engs,
                                       min_val=0, max_val=SLOTS - npart,
                                       skip_runtime_bounds_check=True)
                    f = nc.values_load(mt[0:1, 4 * p + 2 : 4 * p + 3], engines=engs,
                                       min_val=0, max_val=1,
                                       skip_runtime_bounds_check=True)
                    regs.append((qS, s, f))
                    rows = 32 * npart
                    xk3 = xt[rows * u : rows * (u + 1), :].rearrange(
                        "p (a b) -> p a b", b=512)
                    qL.dma_start(xk3, _nslab_ap(o4, s, p, npart), cond=f, cond_hint=True)
                return xt, regs

            def mask_mul(xt, mcol):
                for j in range(HW // mulw):
                    pt = psum.tile([128, mulw], _F32)
                    for b in range(nbank):
                        col = mcol * HW + j * mulw + b * 512
                        nc.tensor.matmul(pt[:, 512 * b : 512 * b + 512], eyes[:],
                                         mpk[:, col : col + 512],
                                         start=True, stop=True)
                    nc.vector.tensor_mul(xt[:, j * mulw : (j + 1) * mulw],
                                         xt[:, j * mulw : (j + 1) * mulw], pt[:])

            def unit_store(xt, regs, it, units, npart):
                for u in range(units):
                    p = it_pos(it, u, units)
                    qS, s, f = regs[u]
                    rows = 32 * npart
                    xk3 = xt[rows * u : rows * (u + 1), :].rearrange(
                        "p (a b) -> p a b", b=512)
                    qS.dma_start(_nslab_ap(o4, s, p, npart), xk3, cond=f, cond_hint=True)

            def it_pos(it, u, units):
                # pair-items occupy meta positions [0, w2*2); single-items after
                return it * 2 + u if units == 2 else w2 * 2 + (it - w2) * 4 + u

            with tc.For_i(0, nc.values_load(rtile[0:1, 0:1]), 1):
                for it in range(w2 + w1):
                    units, npart = (2, 2) if it < w2 else (4, 1)
                    xt, regs = unit_io(it, units, npart)
                    mask_mul(xt, it)
                    unit_store(xt, regs, it, units, npart)

    nc.compile()
    return nc


_MODULES: dict = {}


def _get_module(w_items: int):
    if w_items not in _MODULES:
        _MODULES[w_items] = _build_module(w_items)
    return _MODULES[w_items]


def _host_masks(key_pts: np.ndarray, roll: np.ndarray) -> np.ndarray:
    """Per-(n,g) masks [N, GROUPS, H*W] in {0,1} f32, math exactly as reference."""
    s = int(0.25 * W)
    kx = (key_pts[:, :GROUPS, 0] * np.float32(W)).astype(np.float32)
    ky = (key_pts[:, :GROUPS, 1] * np.float32(H)).astype(np.float32)
    cond = (roll[:, :GROUPS] < np.float32(P_DROP)) & (kx >= 0) & (ky >= 0)

    bx = np.floor(np.maximum(kx - s, np.float32(0.0)))
    ex = np.floor(np.minimum(kx + s, np.float32(W)))
    by = np.floor(np.maximum(ky - s, np.float32(0.0)))
    ey = np.floor(np.minimum(ky + s, np.float32(H)))

    xs = np.arange(W, dtype=np.float32)
    ys = np.arange(H, dtype=np.float32)
    inx = (xs[None, None, :] >= bx[:, :, None]) & (xs[None, None, :] < ex[:, :, None])
    iny = (ys[None, None, :] >= by[:, :, None]) & (ys[None, None, :] < ey[:, :, None])
    box = iny[:, :, :, None] & inx[:, :, None, :]  # [N, G, H, W] bool

    mask = np.where(cond[:, :, None, None], box, True)
    return mask.reshape(N, GROUPS, HW).astype(np.float32)


def build_schedule(key_pts: np.ndarray, roll: np.ndarray):
    """Host schedule: per-core packed work lists from the mask table.

    Returns (w_items, metas [8][1, nslots*4] i32, mpks [8][nslots, 1024] bf16).
    A slot is active iff its mask differs from all-ones; its window chunk c is
    chosen from the mask's nonzero rows so that chunks [c, c+2) cover them.
    """
    masks = _host_masks(key_pts, roll)  # [N, G, HW] f32 0/1
    m_core = masks.reshape(N_CORES, SLOTS, H, W)
    # Exact cover: adjacent masked slots (s, s+1) merge into one 2-slab unit
    # (one 768KB L + one S); isolated masked slots stay 1-slab units.
    pairs_w, singles_w = [], []
    for cidx in range(N_CORES):
        masked = [sl for sl in range(SLOTS) if m_core[cidx, sl].min() < 1.0]
        pairs, singles = [], []
        i = 0
        while i < len(masked):
            if i + 1 < len(masked) and masked[i + 1] == masked[i] + 1:
                pairs.append(masked[i]); i += 2
            else:
                singles.append(masked[i]); i += 1
        pairs_w.append(pairs); singles_w.append(singles)

    w2 = min(16, max(1, max(-(-len(p) // 2) for p in pairs_w)))   # pair-items (2 units)
    w1 = min(16, max(1, max(-(-len(s) // 4) for s in singles_w))) # single-items (4 units)
    npos = w2 * 2 + w1 * 4

    metas, mpks = [], []
    flat = masks.reshape(N_CORES, SLOTS, HW)
    for cidx in range(N_CORES):
        meta = np.zeros((1, npos * 4), dtype=np.int32)
        mpk = np.zeros((4, (w2 + w1) * HW), dtype=np.float32)
        for p, sl in enumerate(pairs_w[cidx]):       # pair positions 0..w2*2
            meta[0, 4 * p : 4 * p + 3] = (sl, 0, 1)
            it, u = divmod(p, 2)
            for j in range(2):
                mpk[2 * u + j, it * HW : (it + 1) * HW] = flat[cidx, sl + j]
        for q, sl in enumerate(singles_w[cidx]):     # single positions after pairs
            p = w2 * 2 + q
            meta[0, 4 * p : 4 * p + 3] = (sl, 0, 1)
            it, k = divmod(q, 4)
            mpk[k, (w2 + it) * HW : (w2 + it + 1) * HW] = flat[cidx, sl]
        metas.append(meta)
        mpks.append(mpk.astype(ml_dtypes.bfloat16))
    return (w2, w1), metas, mpks


def _eyes4() -> np.ndarray:
    e = np.zeros((4, 128), dtype=np.float32)
    for k in range(4):
        e[k, 32 * k : 32 * k + 32] = 1.0
    return e.astype(ml_dtypes.bfloat16)


def make_runner(nc):
    """jit'd shard_map runner over 8 cores with the 'out' buffer donated.

    Returns fn(meta_g, mpk_g, eyes_g, nreps_g, out_g) -> out_g ([512,32,6,512]
    f32 jax array). out_g is consumed (donated); chain calls by passing the
    previous result.
    """
    import jax
    from jax.sharding import Mesh, PartitionSpec
    from jax.experimental.shard_map import shard_map
    from concourse.bass2jax import (
        _bass_exec_p,
        install_neuronx_cc_hook,
        partition_id_tensor,
    )

    install_neuronx_cc_hook()
    partition_name = nc.partition_id_tensor.name if nc.partition_id_tensor else None

    in_names, out_names, out_avals = [], [], []
    for alloc in nc.m.functions[0].allocations:
        if not isinstance(alloc, mybir.MemoryLocationSet):
            continue
        name = alloc.memorylocations[0].name
        if alloc.kind == "ExternalInput":
            if name != partition_name:
                in_names.append(name)
        elif alloc.kind == "ExternalOutput":
            out_names.append(name)
            out_avals.append(jax.core.ShapedArray(tuple(alloc.tensor_shape),
                                                  mybir.dt.np(alloc.dtype)))
    assert out_names == ["out"]
    order = ["meta", "mpk", "eyes", "nreps"]
    assert sorted(in_names) == sorted(order), in_names
    perm = [order.index(n) for n in in_names]
    all_names = tuple(in_names) + ("out",)
    if partition_name is not None:
        all_names = all_names + (partition_name,)

    def _body(*args):
        operands = [args[perm[i]] for i in range(4)] + [args[4]]
        if partition_name is not None:
            operands.append(partition_id_tensor())
        (res,) = _bass_exec_p.bind(
            *operands,
            out_avals=tuple(out_avals),
            in_names=all_names,
            out_names=("out",),
            lowering_input_output_aliases=(),
            sim_require_finite=False, sim_require_nnan=False, nc=nc)
        return (res,)

    mesh = Mesh(np.asarray(jax.devices()[:N_CORES]), ("core",))
    specs = (PartitionSpec("core"),) * 5
    fn = jax.jit(
        shard_map(_body, mesh=mesh, in_specs=specs,
                  out_specs=(PartitionSpec("core"),), check_rep=False),
        donate_argnums=(4,), keep_unused=True)
    return fn, mesh


def kernel(x: np.ndarray, key_pts: np.ndarray, roll: np.ndarray, **_kw) -> np.ndarray:
    import jax
    from jax.sharding import NamedSharding, PartitionSpec

    x = np.ascontiguousarray(np.asarray(x, dtype=np.float32))
    key_pts = np.asarray(key_pts, dtype=np.float32)
    roll = np.asarray(roll, dtype=np.float32)

    w_items, metas, mpks = build_schedule(key_pts, roll)
    nc = _get_module(w_items)
    fn, mesh = make_runner(nc)
    sharding = NamedSharding(mesh, PartitionSpec("core"))

    meta_g = jax.device_put(np.concatenate(metas, axis=0), sharding)
    mpk_g = jax.device_put(np.concatenate(mpks, axis=0), sharding)
    eyes_g = jax.device_put(np.concatenate([_eyes4()] * N_CORES, axis=0), sharding)
    reps_g = jax.device_put(np.ones((N_CORES, 1), np.int32), sharding)
    out_g = jax.device_put(x.reshape(N * GROUPS, CHS, NCHUNK, 512), sharding)

    res = fn(meta_g, mpk_g, eyes_g, reps_g, out_g)[0]
    return np.asarray(res).reshape(N, C, H, W)



# revision 4
# speedup vs baseline: 1.3458x; 1.3458x over previous
"""DropPart masking kernel for Trainium2 (8 NeuronCores, data-parallel over batch).

Problem: x (64, 256, 96, 32) f32. Per sample n and channel-group g (8 groups x
32 channels), a keypoint defines a keep-box; if roll[n,g] < 0.5 the group's
channels are zeroed outside the box (box <= 16x16 in the 96x32 image), else the
group passes through unchanged.

The op multiplies ~half the (n, g) "slots" by a 0/1 mask and leaves the rest
alone. This kernel runs IN-PLACE (the full x tensor is donated as the buffer
backing the NEFF's output), so identity slots cost ZERO HBM traffic, and each
masked slot is updated by a single scatter-multiply: one gpsimd
indirect_dma_start with compute_op=mult streams the slot's precomputed f32
mask rows from SBUF into the DMA engines' CCE (collective-compute ALU), which
reads the slab in HBM, multiplies, and writes it back.  One instruction covers
4 slots (128 indices x one 3072-element channel row each, 1.5 MB); a whole
core's masked work is ~6-9 instructions, so there is no per-slot issue
overhead, no values_load, no PE/DVE work, and no SBUF staging of x at all.

Per-core work lists are data-dependent; the single SPMD program takes a
[128, W] i32 index tensor (row index into the [2048, 3072] slab-row view of
the core's x shard, or an out-of-bounds PAD value) and a [128, W*3072] f32
mask tensor. Padding indices fail the DGE bounds check and are silently
skipped (oob_is_err=False), so each core only moves its own active bytes.
dep_tracking_offset pins each scatter to a disjoint fake region so the Tile
scheduler pipelines them instead of serializing on the whole output tensor.

Masking is idempotent (mask in {0,1}), so the For_i(nreps) timing wrapper can
repeat the body in-place.  Mode "B" (gather -> tensor_mul -> scatter) is kept
as a fallback in case the CCE-mult DMA path misbehaves on hardware.
"""

import os

import numpy as np

import concourse.bass as bass
import concourse.bacc as bacc
import concourse.tile as tile
from concourse import mybir

N, C, H, W = 64, 256, 96, 32
GROUPS = 8
P_DROP = 0.5
HW = H * W              # 3072 elements per channel image
CHS = C // GROUPS       # 32 channels per group
N_CORES = 8
NPC = N // N_CORES      # samples per core = 8
SLOTS = NPC * GROUPS    # (sample, group) slots per core = 64
ROWS = SLOTS * CHS      # 2048 channel rows of 3072 f32 per core
PAD_IDX = ROWS          # out-of-bounds row index -> DGE skips the transfer

MODE = os.environ.get("DROPPART_MODE", "B")

_F32 = mybir.dt.float32
_I32 = mybir.dt.int32


def _host_masks(key_pts: np.ndarray, roll: np.ndarray) -> np.ndarray:
    """Per-(n,g) masks [N, GROUPS, H*W] in {0,1} f32, math exactly as reference."""
    s = int(0.25 * W)
    kx = (key_pts[:, :GROUPS, 0] * np.float32(W)).astype(np.float32)
    ky = (key_pts[:, :GROUPS, 1] * np.float32(H)).astype(np.float32)
    cond = (roll[:, :GROUPS] < np.float32(P_DROP)) & (kx >= 0) & (ky >= 0)

    bx = np.floor(np.maximum(kx - s, np.float32(0.0)))
    ex = np.floor(np.minimum(kx + s, np.float32(W)))
    by = np.floor(np.maximum(ky - s, np.float32(0.0)))
    ey = np.floor(np.minimum(ky + s, np.float32(H)))

    xs = np.arange(W, dtype=np.float32)
    ys = np.arange(H, dtype=np.float32)
    inx = (xs[None, None, :] >= bx[:, :, None]) & (xs[None, None, :] < ex[:, :, None])
    iny = (ys[None, None, :] >= by[:, :, None]) & (ys[None, None, :] < ey[:, :, None])
    box = iny[:, :, :, None] & inx[:, :, None, :]  # [N, G, H, W] bool

    mask = np.where(cond[:, :, None, None], box, True)
    return mask.reshape(N, GROUPS, HW).astype(np.float32)


def build_schedule(key_pts: np.ndarray, roll: np.ndarray):
    """Host schedule: per-core index + mask tensors for the scatter-mults.

    Returns (w_items, idxs [8][128, w_items] i32, mpks [8][128, w_items*HW] f32).
    Item k's partition p covers slot slots[4k + p//32], channel p%32; inactive
    positions carry PAD_IDX and are skipped by the DGE bounds check.
    """
    masks = _host_masks(key_pts, roll)  # [N, G, HW] f32 0/1
    m_core = masks.reshape(N_CORES, SLOTS, HW)
    active = [[sl for sl in range(SLOTS) if m_core[c, sl].min() < 1.0]
              for c in range(N_CORES)]
    w_items = max(1, max(-(-len(a) // 4) for a in active))

    idxs, mpks = [], []
    ch = np.arange(CHS, dtype=np.int32)
    for c in range(N_CORES):
        idx = np.full((128, w_items), PAD_IDX, dtype=np.int32)
        mpk = np.zeros((128, w_items * HW), dtype=np.float32)
        for j, sl in enumerate(active[c]):
            k, q = divmod(j, 4)
            rows = slice(CHS * q, CHS * (q + 1))
            idx[rows, k] = sl * CHS + ch
            mpk[rows, k * HW : (k + 1) * HW] = m_core[c, sl][None, :]
        idxs.append(idx)
        mpks.append(mpk)
    return w_items, idxs, mpks


def _build_module(w_items: int, mode: str = MODE):
    """One SPMD module: per item k, a 128-index indirect DMA touching up to 4
    slots.  Mode "C": scatter-multiply the masks straight onto the HBM slab
    rows (CCE mult in the DMA datapath).  Mode "B": gather rows into SBUF,
    tensor_mul with the mask, scatter back (bypass)."""
    nc = bacc.Bacc("TRN2", target_bir_lowering=False, debug=False)

    o4 = nc.dram_tensor("out", [ROWS, HW], _F32, kind="ExternalOutput").ap()
    idx_d = nc.dram_tensor("idx", [128, w_items], _I32, kind="ExternalInput").ap()
    mpk_d = nc.dram_tensor("mpk", [128, w_items * HW], _F32, kind="ExternalInput").ap()
    reps_d = nc.dram_tensor("nreps", [1, 1], _I32, kind="ExternalInput").ap()

    def o4_fake(k):
        # Full-tensor AP (offset must be 0 for the indirect lowering), but a
        # disjoint fake dep region per item: items touch provably disjoint
        # slots, so dropping the scheduler's whole-tensor serialization is
        # sound; same-k instructions across For_i iterations stay ordered.
        a = o4[:].copy()
        a.dep_tracking_offset = (k + 1) * ROWS * HW
        return a

    with tile.TileContext(nc) as tc:
        with (
            tc.tile_pool(name="consts", bufs=1) as consts,
            tc.tile_pool(name="xpool", bufs=(4 if mode == "B" else 1)) as xpool,
        ):
            it = consts.tile([128, w_items], _I32)
            nc.sync.dma_start(it[:], idx_d[:])
            mt = consts.tile([128, w_items * HW], _F32)
            nc.sync.dma_start(mt[:], mpk_d[:])
            rtile = consts.tile([1, 1], _I32)
            nc.sync.dma_start(rtile[:], reps_d[:])

            with tc.For_i(0, nc.values_load(rtile[0:1, 0:1]), 1):
                for k in range(w_items):
                    ioff = bass.IndirectOffsetOnAxis(ap=it[:, k : k + 1], axis=0)
                    msl = mt[:, k * HW : (k + 1) * HW]
                    if mode == "C":
                        nc.gpsimd.indirect_dma_start(
                            out=o4_fake(k), out_offset=ioff,
                            in_=msl, in_offset=None,
                            bounds_check=ROWS - 1, oob_is_err=False,
                            compute_op=mybir.AluOpType.mult)
                    else:
                        xt = xpool.tile([128, HW], _F32)
                        nc.gpsimd.indirect_dma_start(
                            out=xt[:], out_offset=None,
                            in_=o4_fake(k), in_offset=ioff,
                            bounds_check=ROWS - 1, oob_is_err=False)
                        nc.vector.tensor_mul(xt[:], xt[:], msl)
                        nc.gpsimd.indirect_dma_start(
                            out=o4_fake(k), out_offset=ioff,
                            in_=xt[:], in_offset=None,
                            bounds_check=ROWS - 1, oob_is_err=False)

    nc.compile()
    return nc


_MODULES: dict = {}


def _get_module(w_items: int):
    key = (w_items, MODE)
    if key not in _MODULES:
        _MODULES[key] = _build_module(w_items)
    return _MODULES[key]


def make_runner(nc):
    """jit'd shard_map runner over 8 cores with the 'out' buffer donated.

    Returns fn(idx_g, mpk_g, nreps_g, out_g) -> out_g ([8*2048, 3072] f32
    sharded).  out_g is consumed (donated); chain calls by passing the
    previous result.
    """
    import jax
    from jax.sharding import Mesh, PartitionSpec
    from jax.experimental.shard_map import shard_map
    from concourse.bass2jax import (
        _bass_exec_p,
        install_neuronx_cc_hook,
        partition_id_tensor,
    )

    install_neuronx_cc_hook()
    partition_name = nc.partition_id_tensor.name if nc.partition_id_tensor else None

    in_names, out_names, out_avals = [], [], []
    for alloc in nc.m.functions[0].allocations:
        if not isinstance(alloc, mybir.MemoryLocationSet):
            continue
        name = alloc.memorylocations[0].name
        if alloc.kind == "ExternalInput":
            if name != partition_name:
                in_names.append(name)
        elif alloc.kind == "ExternalOutput":
            out_names.append(name)
            out_avals.append(jax.core.ShapedArray(tuple(alloc.tensor_shape),
                                                  mybir.dt.np(alloc.dtype)))
    assert out_names == ["out"]
    order = ["idx", "mpk", "nreps"]
    assert sorted(in_names) == sorted(order), in_names
    perm = [order.index(n) for n in in_names]
    all_names = tuple(in_names) + ("out",)
    if partition_name is not None:
        all_names = all_names + (partition_name,)

    def _body(*args):
        operands = [args[perm[i]] for i in range(len(order))] + [args[len(order)]]
        if partition_name is not None:
            operands.append(partition_id_tensor())
        (res,) = _bass_exec_p.bind(
            *operands,
            out_avals=tuple(out_avals),
            in_names=all_names,
            out_names=("out",),
            lowering_input_output_aliases=(),
            sim_require_finite=False, sim_require_nnan=False, nc=nc)
        return (res,)

    mesh = Mesh(np.asarray(jax.devices()[:N_CORES]), ("core",))
    specs = (PartitionSpec("core"),) * 4
    fn = jax.jit(
        shard_map(_body, mesh=mesh, in_specs=specs,
                  out_specs=(PartitionSpec("core"),), check_rep=False),
        donate_argnums=(3,), keep_unused=True)
    return fn, mesh


def kernel(x: np.ndarray, key_pts: np.ndarray, roll: np.ndarray, **_kw) -> np.ndarray:
    import jax
    from jax.sharding import NamedSharding, PartitionSpec

    x = np.ascontiguousarray(np.asarray(x, dtype=np.float32))
    key_pts = np.asarray(key_pts, dtype=np.float32)
    roll = np.asarray(roll, dtype=np.float32)

    w_items, idxs, mpks = build_schedule(key_pts, roll)
    nc = _get_module(w_items)
    fn, mesh = make_runner(nc)
    sharding = NamedSharding(mesh, PartitionSpec("core"))

    idx_g = jax.device_put(np.concatenate(idxs, axis=0), sharding)
    mpk_g = jax.device_put(np.concatenate(mpks, axis=0), sharding)
    reps_g = jax.device_put(np.ones((N_CORES, 1), np.int32), sharding)
    out_g = jax.device_put(x.reshape(N_CORES * ROWS, HW), sharding)

    res = fn(idx_g, mpk_g, reps_g, out_g)[0]
    return np.asarray(res).reshape(N, C, H, W)


# revision 7
# speedup vs baseline: 1.6044x; 1.1922x over previous
"""DropPart masking kernel for Trainium2 (8 NeuronCores, data-parallel over batch).

Problem: x (64, 256, 96, 32) f32. Per sample n and channel-group g (8 groups x
32 channels), a keypoint defines a keep-box; if roll[n,g] < 0.5 the group's
channels are zeroed outside the box (box <= 16x16 in the 96x32 image), else the
group passes through unchanged.

The op multiplies ~half the (n, g) "slots" by a 0/1 mask and leaves the rest
alone. This kernel runs IN-PLACE (the full x tensor is donated as the buffer
backing the NEFF's output), so identity slots cost ZERO HBM traffic, and each
masked slot is updated by a single scatter-multiply: one gpsimd
indirect_dma_start with compute_op=mult streams the slot's precomputed f32
mask rows from SBUF into the DMA engines' CCE (collective-compute ALU), which
reads the slab in HBM, multiplies, and writes it back.  One instruction covers
4 slots (128 indices x one 3072-element channel row each, 1.5 MB); a whole
core's masked work is ~6-9 instructions, so there is no per-slot issue
overhead, no values_load, no PE/DVE work, and no SBUF staging of x at all.

Per-core work lists are data-dependent; the single SPMD program takes a
[128, W] i32 index tensor (row index into the [2048, 3072] slab-row view of
the core's x shard, or an out-of-bounds PAD value) and a [128, W*3072] f32
mask tensor. Padding indices fail the DGE bounds check and are silently
skipped (oob_is_err=False), so each core only moves its own active bytes.
dep_tracking_offset pins each scatter to a disjoint fake region so the Tile
scheduler pipelines them instead of serializing on the whole output tensor.

Masking is idempotent (mask in {0,1}), so the For_i(nreps) timing wrapper can
repeat the body in-place.  Mode "B" (gather -> tensor_mul -> scatter) is kept
as a fallback in case the CCE-mult DMA path misbehaves on hardware.
"""

import os

import numpy as np

import concourse.bass as bass
import concourse.bacc as bacc
import concourse.tile as tile
from concourse import mybir

N, C, H, W = 64, 256, 96, 32
GROUPS = 8
P_DROP = 0.5
HW = H * W              # 3072 elements per channel image
CHS = C // GROUPS       # 32 channels per group
N_CORES = 8
NPC = N // N_CORES      # samples per core = 8
SLOTS = NPC * GROUPS    # (sample, group) slots per core = 64
ROWS = SLOTS * CHS      # 2048 channel rows of 3072 f32 per core
PAD_IDX = ROWS          # out-of-bounds row index -> DGE skips the transfer

MODE = os.environ.get("DROPPART_MODE", "B")

_F32 = mybir.dt.float32
_I32 = mybir.dt.int32


def _host_masks(key_pts: np.ndarray, roll: np.ndarray) -> np.ndarray:
    """Per-(n,g) masks [N, GROUPS, H*W] in {0,1} f32, math exactly as reference."""
    s = int(0.25 * W)
    kx = (key_pts[:, :GROUPS, 0] * np.float32(W)).astype(np.float32)
    ky = (key_pts[:, :GROUPS, 1] * np.float32(H)).astype(np.float32)
    cond = (roll[:, :GROUPS] < np.float32(P_DROP)) & (kx >= 0) & (ky >= 0)

    bx = np.floor(np.maximum(kx - s, np.float32(0.0)))
    ex = np.floor(np.minimum(kx + s, np.float32(W)))
    by = np.floor(np.maximum(ky - s, np.float32(0.0)))
    ey = np.floor(np.minimum(ky + s, np.float32(H)))

    xs = np.arange(W, dtype=np.float32)
    ys = np.arange(H, dtype=np.float32)
    inx = (xs[None, None, :] >= bx[:, :, None]) & (xs[None, None, :] < ex[:, :, None])
    iny = (ys[None, None, :] >= by[:, :, None]) & (ys[None, None, :] < ey[:, :, None])
    box = iny[:, :, :, None] & inx[:, :, None, :]  # [N, G, H, W] bool

    mask = np.where(cond[:, :, None, None], box, True)
    return mask.reshape(N, GROUPS, HW).astype(np.float32)


def _balance_perm(counts: np.ndarray) -> np.ndarray:
    """LPT-pack the 64 samples into 8 bins of exactly 8 samples each,
    balancing the per-bin masked-group totals. Returns perm: position i in
    the packed order holds original sample perm[i]; bin c = perm[8c:8c+8]."""
    order = np.argsort(-counts, kind="stable")
    bins = [[] for _ in range(N_CORES)]
    sums = np.zeros(N_CORES)
    for s in order:
        open_bins = [b for b in range(N_CORES) if len(bins[b]) < NPC]
        b = min(open_bins, key=lambda bb: (sums[bb], len(bins[bb])))
        bins[b].append(int(s))
        sums[b] += counts[s]
    return np.array([s for b in bins for s in b], dtype=np.int64)


def build_schedule(key_pts: np.ndarray, roll: np.ndarray):
    """Host schedule: per-core index + mask tensors for the indirect DMAs.

    Samples are permuted (perm) so the per-core masked-slot counts are
    balanced before sharding; the kernel applies the same permutation to x
    and inverts it on the way out.

    Returns (w_items, idxs [8][128, w_items] i32, mpks [8][128, w_items*HW]
    f32, perm [64]).  Item k's partition p covers slot slots[4k + p//32],
    channel p%32; inactive positions carry PAD_IDX and are skipped by the
    DGE bounds check.
    """
    masks = _host_masks(key_pts, roll)  # [N, G, HW] f32 0/1
    masked = masks.min(axis=2) < 1.0  # [N, G] bool
    perm = _balance_perm(masked.sum(axis=1).astype(np.float64))
    m_core = masks[perm].reshape(N_CORES, SLOTS, HW)
    active = [[sl for sl in range(SLOTS) if m_core[c, sl].min() < 1.0]
              for c in range(N_CORES)]
    w_items = max(1, max(-(-len(a) // 4) for a in active))

    idxs, mpks = [], []
    ch = np.arange(CHS, dtype=np.int32)
    for c in range(N_CORES):
        idx = np.full((128, w_items), PAD_IDX, dtype=np.int32)
        mpk = np.zeros((128, w_items * HW), dtype=np.float32)
        for j, sl in enumerate(active[c]):
            k, q = divmod(j, 4)
            rows = slice(CHS * q, CHS * (q + 1))
            idx[rows, k] = sl * CHS + ch
            mpk[rows, k * HW : (k + 1) * HW] = m_core[c, sl][None, :]
        idxs.append(idx)
        mpks.append(mpk)
    return w_items, idxs, mpks, perm


def _build_module(w_items: int, mode: str = MODE):
    """One SPMD module: per item k, a 128-index indirect DMA touching up to 4
    slots.  Mode "C": scatter-multiply the masks straight onto the HBM slab
    rows (CCE mult in the DMA datapath).  Mode "B": gather rows into SBUF,
    tensor_mul with the mask, scatter back (bypass)."""
    nc = bacc.Bacc("TRN2", target_bir_lowering=False, debug=False)

    o4 = nc.dram_tensor("out", [ROWS, HW], _F32, kind="ExternalOutput").ap()
    idx_d = nc.dram_tensor("idx", [128, w_items], _I32, kind="ExternalInput").ap()
    mpk_d = nc.dram_tensor("mpk", [128, w_items * HW], _F32, kind="ExternalInput").ap()
    reps_d = nc.dram_tensor("nreps", [1, 1], _I32, kind="ExternalInput").ap()

    def o4_fake(k):
        # Full-tensor AP (offset must be 0 for the indirect lowering), but a
        # disjoint fake dep region per item: items touch provably disjoint
        # slots, so dropping the scheduler's whole-tensor serialization is
        # sound; same-k instructions across For_i iterations stay ordered.
        a = o4[:].copy()
        a.dep_tracking_offset = (k + 1) * ROWS * HW
        return a

    with tile.TileContext(nc) as tc:
        with (
            tc.tile_pool(name="consts", bufs=1) as consts,
            tc.tile_pool(name="xpool", bufs=(4 if mode == "B" else 1)) as xpool,
        ):
            it = consts.tile([128, w_items], _I32)
            nc.sync.dma_start(it[:], idx_d[:])
            mt = consts.tile([128, w_items * HW], _F32)
            nc.sync.dma_start(mt[:], mpk_d[:])
            rtile = consts.tile([1, 1], _I32)
            nc.sync.dma_start(rtile[:], reps_d[:])

            with tc.For_i(0, nc.values_load(rtile[0:1, 0:1]), 1):
                for k in range(w_items):
                    ioff = bass.IndirectOffsetOnAxis(ap=it[:, k : k + 1], axis=0)
                    msl = mt[:, k * HW : (k + 1) * HW]
                    if mode == "C":
                        nc.gpsimd.indirect_dma_start(
                            out=o4_fake(k), out_offset=ioff,
                            in_=msl, in_offset=None,
                            bounds_check=ROWS - 1, oob_is_err=False,
                            compute_op=mybir.AluOpType.mult)
                    else:
                        xt = xpool.tile([128, HW], _F32)
                        nc.gpsimd.indirect_dma_start(
                            out=xt[:], out_offset=None,
                            in_=o4_fake(k), in_offset=ioff,
                            bounds_check=ROWS - 1, oob_is_err=False)
                        nc.vector.tensor_mul(xt[:], xt[:], msl)
                        nc.gpsimd.indirect_dma_start(
                            out=o4_fake(k), out_offset=ioff,
                            in_=xt[:], in_offset=None,
                            bounds_check=ROWS - 1, oob_is_err=False)

    nc.compile()
    return nc


_MODULES: dict = {}


def _get_module(w_items: int):
    key = (w_items, MODE)
    if key not in _MODULES:
        _MODULES[key] = _build_module(w_items)
    return _MODULES[key]


def make_runner(nc):
    """jit'd shard_map runner over 8 cores with the 'out' buffer donated.

    Returns fn(idx_g, mpk_g, nreps_g, out_g) -> out_g ([8*2048, 3072] f32
    sharded).  out_g is consumed (donated); chain calls by passing the
    previous result.
    """
    import jax
    from jax.sharding import Mesh, PartitionSpec
    from jax.experimental.shard_map import shard_map
    from concourse.bass2jax import (
        _bass_exec_p,
        install_neuronx_cc_hook,
        partition_id_tensor,
    )

    install_neuronx_cc_hook()
    partition_name = nc.partition_id_tensor.name if nc.partition_id_tensor else None

    in_names, out_names, out_avals = [], [], []
    for alloc in nc.m.functions[0].allocations:
        if not isinstance(alloc, mybir.MemoryLocationSet):
            continue
        name = alloc.memorylocations[0].name
        if alloc.kind == "ExternalInput":
            if name != partition_name:
                in_names.append(name)
        elif alloc.kind == "ExternalOutput":
            out_names.append(name)
            out_avals.append(jax.core.ShapedArray(tuple(alloc.tensor_shape),
                                                  mybir.dt.np(alloc.dtype)))
    assert out_names == ["out"]
    order = ["idx", "mpk", "nreps"]
    assert sorted(in_names) == sorted(order), in_names
    perm = [order.index(n) for n in in_names]
    all_names = tuple(in_names) + ("out",)
    if partition_name is not None:
        all_names = all_names + (partition_name,)

    def _body(*args):
        operands = [args[perm[i]] for i in range(len(order))] + [args[len(order)]]
        if partition_name is not None:
            operands.append(partition_id_tensor())
        (res,) = _bass_exec_p.bind(
            *operands,
            out_avals=tuple(out_avals),
            in_names=all_names,
            out_names=("out",),
            lowering_input_output_aliases=(),
            sim_require_finite=False, sim_require_nnan=False, nc=nc)
        return (res,)

    mesh = Mesh(np.asarray(jax.devices()[:N_CORES]), ("core",))
    specs = (PartitionSpec("core"),) * 4
    fn = jax.jit(
        shard_map(_body, mesh=mesh, in_specs=specs,
                  out_specs=(PartitionSpec("core"),), check_rep=False),
        donate_argnums=(3,), keep_unused=True)
    return fn, mesh


def kernel(x: np.ndarray, key_pts: np.ndarray, roll: np.ndarray, **_kw) -> np.ndarray:
    import jax
    from jax.sharding import NamedSharding, PartitionSpec

    x = np.ascontiguousarray(np.asarray(x, dtype=np.float32))
    key_pts = np.asarray(key_pts, dtype=np.float32)
    roll = np.asarray(roll, dtype=np.float32)

    w_items, idxs, mpks, perm = build_schedule(key_pts, roll)
    nc = _get_module(w_items)
    fn, mesh = make_runner(nc)
    sharding = NamedSharding(mesh, PartitionSpec("core"))

    idx_g = jax.device_put(np.concatenate(idxs, axis=0), sharding)
    mpk_g = jax.device_put(np.concatenate(mpks, axis=0), sharding)
    reps_g = jax.device_put(np.ones((N_CORES, 1), np.int32), sharding)
    out_g = jax.device_put(x[perm].reshape(N_CORES * ROWS, HW), sharding)

    res = np.asarray(fn(idx_g, mpk_g, reps_g, out_g)[0]).reshape(N, C, H, W)
    final = np.empty_like(res)
    final[perm] = res
    return final


# revision 15
# speedup vs baseline: 1.7893x; 1.1152x over previous
"""DropPart masking kernel for Trainium2 (8 NeuronCores, data-parallel over batch).

Problem: x (64, 256, 96, 32) f32. Per sample n and channel-group g (8 groups x
32 channels), a keypoint defines a keep-box; if roll[n,g] < 0.5 the group's
channels are zeroed outside the box (box <= 16x16 in the 96x32 image), else the
group passes through unchanged.

The op multiplies ~half the (n, g) "slots" by a 0/1 mask and leaves the rest
alone.  This kernel runs IN-PLACE (the full x tensor is donated as the buffer
backing the NEFF's output), so identity slots cost ZERO HBM traffic.  All
masked-slot I/O goes through batched gpsimd indirect DMAs (gather/scatter with
an SBUF index tile, one DRAM row per index), so a core's whole data-dependent
work list is a handful of instructions with no per-slot issue overhead and no
values_load plumbing; inactive index positions carry an out-of-bounds PAD
value that the DGE bounds check silently skips, so padding costs nothing.

Host-side, samples are permuted so the per-core masked-slot counts are
balanced (the straggler core sets the kernel time), and the permutation is
inverted on the way out.

Mode "W" (default) exploits the mask structure: the keep-box spans at most 16
of the 96 image rows, i.e. at most 2 of the 6 512-element chunks per channel
row.  Per (slot, channel): gather only the <=2 box-intersecting chunks
([128, 2]-index gather, PAD for single-chunk boxes), multiply by the matching
window mask, scatter back; the remaining 4-5 chunks are pure zeros in the
output and are written by a zero-chunk scatter from a static zero tile
([128, 5] indices).  Window rows and zero rows are disjoint by construction,
so the three DMAs per item have no ordering hazards; ~480KB moves per masked
slot instead of the 768KB of a full slab read-modify-write.

Mode "B" (fallback): full-slab RMW -- gather 4 slots' 32 channel rows
([128, 3072] f32, 1.5MB), tensor_mul with a resident mask, scatter back.

dep_tracking_offset pins each indirect DMA to a disjoint fake region so the
Tile scheduler pipelines instructions instead of serializing on the whole
output tensor; items touch provably disjoint slots, so this is sound, and
same-region instructions across For_i iterations stay ordered.  Masking is
idempotent (mask in {0,1}), so the For_i(nreps) timing wrapper can repeat the
body in-place.
"""

import os

import numpy as np

import concourse.bass as bass
import concourse.bacc as bacc
import concourse.tile as tile
from concourse import mybir

N, C, H, W = 64, 256, 96, 32
GROUPS = 8
P_DROP = 0.5
HW = H * W              # 3072 elements per channel image
CHS = C // GROUPS       # 32 channels per group
N_CORES = 8
NPC = N // N_CORES      # samples per core = 8
SLOTS = NPC * GROUPS    # (sample, group) slots per core = 64
ROWS = SLOTS * CHS      # 2048 channel rows of 3072 f32 per core
PAD_IDX = ROWS          # out-of-bounds row index -> DGE skips the transfer
NCHUNK = 6              # 512-element chunks per channel image (16 rows each)
ROWS6 = ROWS * NCHUNK   # 12288 chunk rows of 512 f32 per core
PAD6 = ROWS6            # out-of-bounds chunk-row index

MODE = os.environ.get("DROPPART_MODE", "W")

_F32 = mybir.dt.float32
_I32 = mybir.dt.int32


def _host_masks(key_pts: np.ndarray, roll: np.ndarray) -> np.ndarray:
    """Per-(n,g) masks [N, GROUPS, H*W] in {0,1} f32, math exactly as reference."""
    s = int(0.25 * W)
    kx = (key_pts[:, :GROUPS, 0] * np.float32(W)).astype(np.float32)
    ky = (key_pts[:, :GROUPS, 1] * np.float32(H)).astype(np.float32)
    cond = (roll[:, :GROUPS] < np.float32(P_DROP)) & (kx >= 0) & (ky >= 0)

    bx = np.floor(np.maximum(kx - s, np.float32(0.0)))
    ex = np.floor(np.minimum(kx + s, np.float32(W)))
    by = np.floor(np.maximum(ky - s, np.float32(0.0)))
    ey = np.floor(np.minimum(ky + s, np.float32(H)))

    xs = np.arange(W, dtype=np.float32)
    ys = np.arange(H, dtype=np.float32)
    inx = (xs[None, None, :] >= bx[:, :, None]) & (xs[None, None, :] < ex[:, :, None])
    iny = (ys[None, None, :] >= by[:, :, None]) & (ys[None, None, :] < ey[:, :, None])
    box = iny[:, :, :, None] & inx[:, :, None, :]  # [N, G, H, W] bool

    mask = np.where(cond[:, :, None, None], box, True)
    return mask.reshape(N, GROUPS, HW).astype(np.float32)


def _balance_perm(counts: np.ndarray) -> np.ndarray:
    """LPT-pack the 64 samples into 8 bins of exactly 8 samples each,
    balancing the per-bin masked-group totals. Returns perm: position i in
    the packed order holds original sample perm[i]; bin c = perm[8c:8c+8]."""
    order = np.argsort(-counts, kind="stable")
    bins = [[] for _ in range(N_CORES)]
    sums = np.zeros(N_CORES)
    for s in order:
        open_bins = [b for b in range(N_CORES) if len(bins[b]) < NPC]
        b = min(open_bins, key=lambda bb: (sums[bb], len(bins[bb])))
        bins[b].append(int(s))
        sums[b] += counts[s]
    return np.array([s for b in bins for s in b], dtype=np.int64)


def build_schedule(key_pts: np.ndarray, roll: np.ndarray):
    """Host schedule for the indirect DMAs.

    Returns (w_items, ins: dict name -> per-core list of host arrays, perm).
    Item k's partition p covers slot active[4k + p//32], channel p%32.
    """
    masks = _host_masks(key_pts, roll)  # [N, G, HW] f32 0/1
    masked = masks.min(axis=2) < 1.0  # [N, G] bool
    perm = _balance_perm(masked.sum(axis=1).astype(np.float64))
    m_core = masks[perm].reshape(N_CORES, SLOTS, HW)
    active = [[sl for sl in range(SLOTS) if m_core[c, sl].min() < 1.0]
              for c in range(N_CORES)]
    w_items = max(1, max(-(-len(a) // 4) for a in active))

    ch = np.arange(CHS, dtype=np.int32)
    if MODE == "B":
        idxs, mpks = [], []
        for c in range(N_CORES):
            idx = np.full((128, w_items), PAD_IDX, dtype=np.int32)
            mpk = np.zeros((128, w_items * HW), dtype=np.float32)
            for j, sl in enumerate(active[c]):
                k, q = divmod(j, 4)
                rows = slice(CHS * q, CHS * (q + 1))
                idx[rows, k] = sl * CHS + ch
                mpk[rows, k * HW : (k + 1) * HW] = m_core[c, sl][None, :]
            idxs.append(idx)
            mpks.append(mpk)
        return w_items, {"idx": idxs, "mpk": mpks}, perm

    # MODE "W": per (slot, channel), a CONSECUTIVE pair of window chunks
    # (covers the <=16-row box; the HW DGE auto-increments multi-index
    # transfers from idx[p,0], so indices within an instruction must be
    # consecutive -- host writes the actual consecutive values so the
    # functional interpreter agrees), plus a full 6-chunk zero run.  Order
    # per item: gather windows, zero the slabs, scatter the multiplied
    # windows back (same dep region serializes the three DMAs).
    idxws, idxzs, wmpks = [], [], []
    for c in range(N_CORES):
        idxw = np.full((128, 2 * w_items), PAD6, dtype=np.int32)
        idxz = np.full((128, 6 * w_items), PAD6, dtype=np.int32)
        wmpk = np.zeros((128, w_items * 1024), dtype=np.float32)
        for j, sl in enumerate(active[c]):
            k, q = divmod(j, 4)
            rows = slice(CHS * q, CHS * (q + 1))
            m6 = m_core[c, sl].reshape(NCHUNK, 512)
            nzc = np.nonzero(m6.any(axis=1))[0]
            assert 1 <= len(nzc) <= 2 and nzc[-1] - nzc[0] == len(nzc) - 1, nzc
            c0 = min(int(nzc[0]), NCHUNK - 2)  # window pair [c0, c0+1]
            base = (sl * CHS + ch) * NCHUNK  # chunk-row base per channel
            for b in range(2):
                idxw[rows, 2 * k + b] = base + c0 + b
                wmpk[rows, k * 1024 + 512 * b : k * 1024 + 512 * (b + 1)] = \
                    m6[c0 + b][None, :]
            for z in range(NCHUNK):
                idxz[rows, 6 * k + z] = base + z
        idxws.append(idxw)
        idxzs.append(idxz)
        wmpks.append(wmpk)
    return w_items, {"idxw": idxws, "idxz": idxzs, "wmpk": wmpks}, perm


def _build_module(w_items: int, mode: str = MODE):
    """One SPMD module; all data movement via 128-index indirect DMAs."""
    nc = bacc.Bacc("TRN2", target_bir_lowering=False, debug=False)

    reps_d = nc.dram_tensor("nreps", [1, 1], _I32, kind="ExternalInput").ap()
    if mode == "B":
        o_d = nc.dram_tensor("out", [ROWS, HW], _F32, kind="ExternalOutput").ap()
        idx_d = nc.dram_tensor("idx", [128, w_items], _I32, kind="ExternalInput").ap()
        mpk_d = nc.dram_tensor("mpk", [128, w_items * HW], _F32, kind="ExternalInput").ap()
        nrow, pad = ROWS, PAD_IDX
    else:
        o_d = nc.dram_tensor("out", [ROWS6, 512], _F32, kind="ExternalOutput").ap()
        idxw_d = nc.dram_tensor("idxw", [128, 2 * w_items], _I32, kind="ExternalInput").ap()
        idxz_d = nc.dram_tensor("idxz", [128, 6 * w_items], _I32, kind="ExternalInput").ap()
        wmpk_d = nc.dram_tensor("wmpk", [128, w_items * 1024], _F32, kind="ExternalInput").ap()
        nrow, pad = ROWS6, PAD6

    def o_fake(r):
        # Full-tensor AP (offset must be 0 for the indirect lowering), but a
        # disjoint fake dep region per item: items touch provably disjoint
        # rows, so dropping the scheduler's whole-tensor serialization is
        # sound; same-region instructions across For_i iterations stay
        # ordered.
        a = o_d[:].copy()
        a.dep_tracking_offset = (r + 1) * nrow * (HW if mode == "B" else 512)
        return a

    with tile.TileContext(nc) as tc:
        with (
            tc.tile_pool(name="consts", bufs=1) as consts,
            tc.tile_pool(name="xpool", bufs=4) as xpool,
        ):
            rtile = consts.tile([1, 1], _I32)
            nc.sync.dma_start(rtile[:], reps_d[:])
            if mode == "B":
                it = consts.tile([128, w_items], _I32)
                nc.sync.dma_start(it[:], idx_d[:])
                mt = consts.tile([128, w_items * HW], _F32)
                nc.sync.dma_start(mt[:], mpk_d[:])
            else:
                # The DGE misreads sliced offset APs, so each indirect DMA
                # gets its own full [128, B] index tile.
                itws, itzs = [], []
                for k in range(w_items):
                    itw = consts.tile([128, 2], _I32, name=f"itw{k}")
                    nc.sync.dma_start(itw[:], idxw_d[:, 2 * k : 2 * (k + 1)])
                    itws.append(itw)
                    itz = consts.tile([128, 6], _I32, name=f"itz{k}")
                    nc.scalar.dma_start(itz[:], idxz_d[:, 6 * k : 6 * (k + 1)])
                    itzs.append(itz)
                wmt = consts.tile([128, w_items * 1024], _F32)
                nc.sync.dma_start(wmt[:], wmpk_d[:])
                zt = consts.tile([128, 6 * 512], _F32)
                nc.vector.memset(zt[:], 0.0)

            with tc.For_i(0, nc.values_load(rtile[0:1, 0:1]), 1):
                for k in range(w_items):
                    if mode == "B":
                        ioff = bass.IndirectOffsetOnAxis(ap=it[:, k : k + 1], axis=0)
                        xt = xpool.tile([128, HW], _F32)
                        nc.gpsimd.indirect_dma_start(
                            out=xt[:], out_offset=None,
                            in_=o_fake(k), in_offset=ioff,
                            bounds_check=nrow - 1, oob_is_err=False)
                        nc.vector.tensor_mul(xt[:], xt[:], mt[:, k * HW : (k + 1) * HW])
                        nc.gpsimd.indirect_dma_start(
                            out=o_fake(k), out_offset=ioff,
                            in_=xt[:], in_offset=None,
                            bounds_check=nrow - 1, oob_is_err=False)
                    else:
                        # gather windows -> zero the whole slabs -> scatter
                        # the multiplied windows back; all three share dep
                        # region k, so Tile serializes them in issue order.
                        woff = bass.IndirectOffsetOnAxis(ap=itws[k][:], axis=0)
                        wt = xpool.tile([128, 1024], _F32)
                        nc.gpsimd.indirect_dma_start(
                            out=wt[:], out_offset=None,
                            in_=o_fake(k), in_offset=woff,
                            bounds_check=nrow - 1, oob_is_err=False)
                        zoff = bass.IndirectOffsetOnAxis(ap=itzs[k][:], axis=0)
                        nc.gpsimd.indirect_dma_start(
                            out=o_fake(k), out_offset=zoff,
                            in_=zt[:], in_offset=None,
                            bounds_check=nrow - 1, oob_is_err=False)
                        nc.vector.tensor_mul(wt[:], wt[:],
                                             wmt[:, k * 1024 : (k + 1) * 1024])
                        nc.gpsimd.indirect_dma_start(
                            out=o_fake(k), out_offset=woff,
                            in_=wt[:], in_offset=None,
                            bounds_check=nrow - 1, oob_is_err=False)

    nc.compile()
    return nc


_MODULES: dict = {}


def _get_module(w_items: int):
    key = (w_items, MODE)
    if key not in _MODULES:
        _MODULES[key] = _build_module(w_items)
    return _MODULES[key]


def make_runner(nc):
    """jit'd shard_map runner over 8 cores with the 'out' buffer donated.

    Returns (fn, mesh, order): fn(*ins_in_order, out_g) -> (out_g,); out_g is
    consumed (donated); chain calls by passing the previous result.
    """
    import jax
    from jax.sharding import Mesh, PartitionSpec
    from jax.experimental.shard_map import shard_map
    from concourse.bass2jax import (
        _bass_exec_p,
        install_neuronx_cc_hook,
        partition_id_tensor,
    )

    install_neuronx_cc_hook()
    partition_name = nc.partition_id_tensor.name if nc.partition_id_tensor else None

    in_names, out_names, out_avals = [], [], []
    for alloc in nc.m.functions[0].allocations:
        if not isinstance(alloc, mybir.MemoryLocationSet):
            continue
        name = alloc.memorylocations[0].name
        if alloc.kind == "ExternalInput":
            if name != partition_name:
                in_names.append(name)
        elif alloc.kind == "ExternalOutput":
            out_names.append(name)
            out_avals.append(jax.core.ShapedArray(tuple(alloc.tensor_shape),
                                                  mybir.dt.np(alloc.dtype)))
    assert out_names == ["out"]
    n_in = len(in_names)
    all_names = tuple(in_names) + ("out",)
    if partition_name is not None:
        all_names = all_names + (partition_name,)

    def _body(*args):
        operands = list(args[:n_in + 1])
        if partition_name is not None:
            operands.append(partition_id_tensor())
        (res,) = _bass_exec_p.bind(
            *operands,
            out_avals=tuple(out_avals),
            in_names=all_names,
            out_names=("out",),
            lowering_input_output_aliases=(),
            sim_require_finite=False, sim_require_nnan=False, nc=nc)
        return (res,)

    mesh = Mesh(np.asarray(jax.devices()[:N_CORES]), ("core",))
    specs = (PartitionSpec("core"),) * (n_in + 1)
    fn = jax.jit(
        shard_map(_body, mesh=mesh, in_specs=specs,
                  out_specs=(PartitionSpec("core"),), check_rep=False),
        donate_argnums=(n_in,), keep_unused=True)
    return fn, mesh, list(in_names)


def device_inputs(ins: dict, mesh, nreps: int = 1):
    """device_put the per-core host arrays (plus the nreps scalar)."""
    import jax
    from jax.sharding import NamedSharding, PartitionSpec

    sharding = NamedSharding(mesh, PartitionSpec("core"))
    d = {name: jax.device_put(np.concatenate(arrs, axis=0), sharding)
         for name, arrs in ins.items()}
    d["nreps"] = jax.device_put(np.full((N_CORES, 1), nreps, np.int32), sharding)
    return d, sharding


def kernel(x: np.ndarray, key_pts: np.ndarray, roll: np.ndarray, **_kw) -> np.ndarray:
    import jax

    x = np.ascontiguousarray(np.asarray(x, dtype=np.float32))
    key_pts = np.asarray(key_pts, dtype=np.float32)
    roll = np.asarray(roll, dtype=np.float32)

    w_items, ins, perm = build_schedule(key_pts, roll)
    nc = _get_module(w_items)
    fn, mesh, order = make_runner(nc)
    d, sharding = device_inputs(ins, mesh)
    out_g = jax.device_put(x[perm].reshape(nc_out_shape()), sharding)

    res = fn(*[d[n] for n in order], out_g)[0]
    res = np.asarray(res).reshape(N, C, H, W)
    final = np.empty_like(res)
    final[perm] = res
    return final


def nc_out_shape():
    return (N_CORES * (ROWS if MODE == "B" else ROWS6), HW if MODE == "B" else 512)


# revision 18
# speedup vs baseline: 1.9751x; 1.1039x over previous
"""DropPart masking kernel for Trainium2 (8 NeuronCores, data-parallel over batch).

Problem: x (64, 256, 96, 32) f32. Per sample n and channel-group g (8 groups x
32 channels), a keypoint defines a keep-box; if roll[n,g] < 0.5 the group's
channels are zeroed outside the box (box <= 16x16 in the 96x32 image), else the
group passes through unchanged.

The op multiplies ~half the (n, g) "slots" by a 0/1 mask and leaves the rest
alone.  This kernel runs IN-PLACE (the full x tensor is donated as the buffer
backing the NEFF's output), so identity slots cost ZERO HBM traffic.  All
masked-slot I/O goes through batched gpsimd indirect DMAs (gather/scatter with
an SBUF index tile, one DRAM row per index), so a core's whole data-dependent
work list is a handful of instructions with no per-slot issue overhead and no
values_load plumbing; inactive index positions carry an out-of-bounds PAD
value that the DGE bounds check silently skips, so padding costs nothing.

Host-side, samples are permuted so the per-core masked-slot counts are
balanced (the straggler core sets the kernel time), and the permutation is
inverted on the way out.

Mode "W" (default) exploits the mask structure: the keep-box spans at most 16
of the 96 image rows, i.e. at most 2 of the 6 512-element chunks per channel
row.  Per (slot, channel): gather only the <=2 box-intersecting chunks
([128, 2]-index gather, PAD for single-chunk boxes), multiply by the matching
window mask, scatter back; the remaining 4-5 chunks are pure zeros in the
output and are written by a zero-chunk scatter from a static zero tile
([128, 5] indices).  Window rows and zero rows are disjoint by construction,
so the three DMAs per item have no ordering hazards; ~480KB moves per masked
slot instead of the 768KB of a full slab read-modify-write.

Mode "B" (fallback): full-slab RMW -- gather 4 slots' 32 channel rows
([128, 3072] f32, 1.5MB), tensor_mul with a resident mask, scatter back.

dep_tracking_offset pins each indirect DMA to a disjoint fake region so the
Tile scheduler pipelines instructions instead of serializing on the whole
output tensor; items touch provably disjoint slots, so this is sound, and
same-region instructions across For_i iterations stay ordered.  Masking is
idempotent (mask in {0,1}), so the For_i(nreps) timing wrapper can repeat the
body in-place.
"""

import os

import numpy as np

import concourse.bass as bass
import concourse.bacc as bacc
import concourse.tile as tile
from concourse import mybir

N, C, H, W = 64, 256, 96, 32
GROUPS = 8
P_DROP = 0.5
HW = H * W              # 3072 elements per channel image
CHS = C // GROUPS       # 32 channels per group
N_CORES = 8
NPC = N // N_CORES      # samples per core = 8
SLOTS = NPC * GROUPS    # (sample, group) slots per core = 64
ROWS = SLOTS * CHS      # 2048 channel rows of 3072 f32 per core
PAD_IDX = ROWS          # out-of-bounds row index -> DGE skips the transfer
NCHUNK = 6              # 512-element chunks per channel image (16 rows each)
ROWS6 = ROWS * NCHUNK   # 12288 chunk rows of 512 f32 per core
PAD6 = ROWS6            # out-of-bounds chunk-row index

MODE = os.environ.get("DROPPART_MODE", "W")

_F32 = mybir.dt.float32
_I32 = mybir.dt.int32


def _host_masks(key_pts: np.ndarray, roll: np.ndarray) -> np.ndarray:
    """Per-(n,g) masks [N, GROUPS, H*W] in {0,1} f32, math exactly as reference."""
    s = int(0.25 * W)
    kx = (key_pts[:, :GROUPS, 0] * np.float32(W)).astype(np.float32)
    ky = (key_pts[:, :GROUPS, 1] * np.float32(H)).astype(np.float32)
    cond = (roll[:, :GROUPS] < np.float32(P_DROP)) & (kx >= 0) & (ky >= 0)

    bx = np.floor(np.maximum(kx - s, np.float32(0.0)))
    ex = np.floor(np.minimum(kx + s, np.float32(W)))
    by = np.floor(np.maximum(ky - s, np.float32(0.0)))
    ey = np.floor(np.minimum(ky + s, np.float32(H)))

    xs = np.arange(W, dtype=np.float32)
    ys = np.arange(H, dtype=np.float32)
    inx = (xs[None, None, :] >= bx[:, :, None]) & (xs[None, None, :] < ex[:, :, None])
    iny = (ys[None, None, :] >= by[:, :, None]) & (ys[None, None, :] < ey[:, :, None])
    box = iny[:, :, :, None] & inx[:, :, None, :]  # [N, G, H, W] bool

    mask = np.where(cond[:, :, None, None], box, True)
    return mask.reshape(N, GROUPS, HW).astype(np.float32)


def _balance_perm(counts: np.ndarray) -> np.ndarray:
    """LPT-pack the 64 samples into 8 bins of exactly 8 samples each,
    balancing the per-bin masked-group totals. Returns perm: position i in
    the packed order holds original sample perm[i]; bin c = perm[8c:8c+8]."""
    order = np.argsort(-counts, kind="stable")
    bins = [[] for _ in range(N_CORES)]
    sums = np.zeros(N_CORES)
    for s in order:
        open_bins = [b for b in range(N_CORES) if len(bins[b]) < NPC]
        b = min(open_bins, key=lambda bb: (sums[bb], len(bins[bb])))
        bins[b].append(int(s))
        sums[b] += counts[s]
    return np.array([s for b in bins for s in b], dtype=np.int64)


def build_schedule(key_pts: np.ndarray, roll: np.ndarray):
    """Host schedule for the indirect DMAs.

    Returns (w_items, ins: dict name -> per-core list of host arrays, perm).
    Item k's partition p covers slot active[4k + p//32], channel p%32.
    """
    masks = _host_masks(key_pts, roll)  # [N, G, HW] f32 0/1
    masked = masks.min(axis=2) < 1.0  # [N, G] bool
    perm = _balance_perm(masked.sum(axis=1).astype(np.float64))
    m_core = masks[perm].reshape(N_CORES, SLOTS, HW)
    active = [[sl for sl in range(SLOTS) if m_core[c, sl].min() < 1.0]
              for c in range(N_CORES)]
    w_items = max(1, max(-(-len(a) // 4) for a in active))

    ch = np.arange(CHS, dtype=np.int32)
    if MODE == "B":
        idxs, mpks = [], []
        for c in range(N_CORES):
            idx = np.full((128, w_items), PAD_IDX, dtype=np.int32)
            mpk = np.zeros((128, w_items * HW), dtype=np.float32)
            for j, sl in enumerate(active[c]):
                k, q = divmod(j, 4)
                rows = slice(CHS * q, CHS * (q + 1))
                idx[rows, k] = sl * CHS + ch
                mpk[rows, k * HW : (k + 1) * HW] = m_core[c, sl][None, :]
            idxs.append(idx)
            mpks.append(mpk)
        return w_items, {"idx": idxs, "mpk": mpks}, perm

    # MODE "W": per (slot, channel), a CONSECUTIVE pair of window chunks
    # (covers the <=16-row box; the HW DGE auto-increments multi-index
    # transfers from idx[p,0], so indices within an instruction must be
    # consecutive -- host writes the actual consecutive values so the
    # functional interpreter agrees), plus a full 6-chunk zero run.  Order
    # per item: gather windows, zero the slabs, scatter the multiplied
    # windows back (same dep region serializes the three DMAs).
    idxws, idxzs, wmpks = [], [], []
    for c in range(N_CORES):
        idxw = np.full((128, 2 * w_items), PAD6, dtype=np.int32)
        idxz = np.full((128, 6 * w_items), PAD6, dtype=np.int32)
        wmpk = np.zeros((128, w_items * 1024), dtype=np.float32)
        for j, sl in enumerate(active[c]):
            k, q = divmod(j, 4)
            rows = slice(CHS * q, CHS * (q + 1))
            m6 = m_core[c, sl].reshape(NCHUNK, 512)
            nzc = np.nonzero(m6.any(axis=1))[0]
            assert 1 <= len(nzc) <= 2 and nzc[-1] - nzc[0] == len(nzc) - 1, nzc
            c0 = min(int(nzc[0]), NCHUNK - 2)  # window pair [c0, c0+1]
            base = (sl * CHS + ch) * NCHUNK  # chunk-row base per channel
            for b in range(2):
                idxw[rows, 2 * k + b] = base + c0 + b
                wmpk[rows, k * 1024 + 512 * b : k * 1024 + 512 * (b + 1)] = \
                    m6[c0 + b][None, :]
            for z in range(NCHUNK):
                idxz[rows, 6 * k + z] = base + z
        idxws.append(idxw)
        idxzs.append(idxz)
        wmpks.append(wmpk)
    return w_items, {"idxw": idxws, "idxz": idxzs, "wmpk": wmpks}, perm


def _build_module(w_items: int, mode: str = MODE):
    """One SPMD module; all data movement via 128-index indirect DMAs."""
    nc = bacc.Bacc("TRN2", target_bir_lowering=False, debug=False)

    reps_d = nc.dram_tensor("nreps", [1, 1], _I32, kind="ExternalInput").ap()
    if mode == "B":
        o_d = nc.dram_tensor("out", [ROWS, HW], _F32, kind="ExternalOutput").ap()
        idx_d = nc.dram_tensor("idx", [128, w_items], _I32, kind="ExternalInput").ap()
        mpk_d = nc.dram_tensor("mpk", [128, w_items * HW], _F32, kind="ExternalInput").ap()
        nrow, pad = ROWS, PAD_IDX
    else:
        o_d = nc.dram_tensor("out", [ROWS6, 512], _F32, kind="ExternalOutput").ap()
        idxw_d = nc.dram_tensor("idxw", [128, 2 * w_items], _I32, kind="ExternalInput").ap()
        idxz_d = nc.dram_tensor("idxz", [128, 6 * w_items], _I32, kind="ExternalInput").ap()
        wmpk_d = nc.dram_tensor("wmpk", [128, w_items * 1024], _F32, kind="ExternalInput").ap()
        nrow, pad = ROWS6, PAD6

    def o_fake(r):
        # Full-tensor AP (offset must be 0 for the indirect lowering), but a
        # disjoint fake dep region per item: items touch provably disjoint
        # rows, so dropping the scheduler's whole-tensor serialization is
        # sound; same-region instructions across For_i iterations stay
        # ordered.
        a = o_d[:].copy()
        a.dep_tracking_offset = (r + 1) * nrow * (HW if mode == "B" else 512)
        return a

    with tile.TileContext(nc) as tc:
        with (
            tc.tile_pool(name="consts", bufs=1) as consts,
            tc.tile_pool(name="xpool", bufs=(4 if mode == "B" else 1)) as xpool,
        ):
            rtile = consts.tile([1, 1], _I32)
            nc.sync.dma_start(rtile[:], reps_d[:])
            if mode == "B":
                it = consts.tile([128, w_items], _I32)
                nc.sync.dma_start(it[:], idx_d[:])
                mt = consts.tile([128, w_items * HW], _F32)
                nc.sync.dma_start(mt[:], mpk_d[:])
            else:
                # The DGE misreads sliced offset APs, so each indirect DMA
                # gets its own full [128, B] index tile.
                itws, itzs = [], []
                for k in range(w_items):
                    itw = consts.tile([128, 2], _I32, name=f"itw{k}")
                    nc.sync.dma_start(itw[:], idxw_d[:, 2 * k : 2 * (k + 1)])
                    itws.append(itw)
                    itz = consts.tile([128, 6], _I32, name=f"itz{k}")
                    nc.scalar.dma_start(itz[:], idxz_d[:, 6 * k : 6 * (k + 1)])
                    itzs.append(itz)
                wmt = consts.tile([128, w_items * 1024], _F32)
                nc.sync.dma_start(wmt[:], wmpk_d[:])
                zt = consts.tile([128, 6 * 512], _F32)
                nc.vector.memset(zt[:], 0.0)

            with tc.For_i(0, nc.values_load(rtile[0:1, 0:1]), 1):
                if mode == "B":
                    for k in range(w_items):
                        ioff = bass.IndirectOffsetOnAxis(ap=it[:, k : k + 1], axis=0)
                        xt = xpool.tile([128, HW], _F32)
                        nc.gpsimd.indirect_dma_start(
                            out=xt[:], out_offset=None,
                            in_=o_fake(k), in_offset=ioff,
                            bounds_check=nrow - 1, oob_is_err=False)
                        nc.vector.tensor_mul(xt[:], xt[:], mt[:, k * HW : (k + 1) * HW])
                        nc.gpsimd.indirect_dma_start(
                            out=o_fake(k), out_offset=ioff,
                            in_=xt[:], in_offset=None,
                            bounds_check=nrow - 1, oob_is_err=False)
                else:
                    # Three phases: all window gathers, then all slab zeros
                    # (+ muls on DVE), then all window scatters.  Item k's
                    # three DMAs share dep region k (serialized in this
                    # order); phase-majoring keeps the in-order Pool queue
                    # from stalling on a region wait while later items'
                    # instructions could already be generating/transferring.
                    wts = []
                    for k in range(w_items):
                        woff = bass.IndirectOffsetOnAxis(ap=itws[k][:], axis=0)
                        wt = xpool.tile([128, 1024], _F32, name=f"wt{k}")
                        nc.gpsimd.indirect_dma_start(
                            out=wt[:], out_offset=None,
                            in_=o_fake(k), in_offset=woff,
                            bounds_check=nrow - 1, oob_is_err=False)
                        wts.append(wt)
                    for k in range(w_items):
                        zoff = bass.IndirectOffsetOnAxis(ap=itzs[k][:], axis=0)
                        nc.gpsimd.indirect_dma_start(
                            out=o_fake(k), out_offset=zoff,
                            in_=zt[:], in_offset=None,
                            bounds_check=nrow - 1, oob_is_err=False)
                        nc.vector.tensor_mul(wts[k][:], wts[k][:],
                                             wmt[:, k * 1024 : (k + 1) * 1024])
                    for k in range(w_items):
                        woff = bass.IndirectOffsetOnAxis(ap=itws[k][:], axis=0)
                        nc.gpsimd.indirect_dma_start(
                            out=o_fake(k), out_offset=woff,
                            in_=wts[k][:], in_offset=None,
                            bounds_check=nrow - 1, oob_is_err=False)

    nc.compile()
    return nc


_MODULES: dict = {}


def _get_module(w_items: int):
    key = (w_items, MODE)
    if key not in _MODULES:
        _MODULES[key] = _build_module(w_items)
    return _MODULES[key]


def make_runner(nc):
    """jit'd shard_map runner over 8 cores with the 'out' buffer donated.

    Returns (fn, mesh, order): fn(*ins_in_order, out_g) -> (out_g,); out_g is
    consumed (donated); chain calls by passing the previous result.
    """
    import jax
    from jax.sharding import Mesh, PartitionSpec
    from jax.experimental.shard_map import shard_map
    from concourse.bass2jax import (
        _bass_exec_p,
        install_neuronx_cc_hook,
        partition_id_tensor,
    )

    install_neuronx_cc_hook()
    partition_name = nc.partition_id_tensor.name if nc.partition_id_tensor else None

    in_names, out_names, out_avals = [], [], []
    for alloc in nc.m.functions[0].allocations:
        if not isinstance(alloc, mybir.MemoryLocationSet):
            continue
        name = alloc.memorylocations[0].name
        if alloc.kind == "ExternalInput":
            if name != partition_name:
                in_names.append(name)
        elif alloc.kind == "ExternalOutput":
            out_names.append(name)
            out_avals.append(jax.core.ShapedArray(tuple(alloc.tensor_shape),
                                                  mybir.dt.np(alloc.dtype)))
    assert out_names == ["out"]
    n_in = len(in_names)
    all_names = tuple(in_names) + ("out",)
    if partition_name is not None:
        all_names = all_names + (partition_name,)

    def _body(*args):
        operands = list(args[:n_in + 1])
        if partition_name is not None:
            operands.append(partition_id_tensor())
        (res,) = _bass_exec_p.bind(
            *operands,
            out_avals=tuple(out_avals),
            in_names=all_names,
            out_names=("out",),
            lowering_input_output_aliases=(),
            sim_require_finite=False, sim_require_nnan=False, nc=nc)
        return (res,)

    mesh = Mesh(np.asarray(jax.devices()[:N_CORES]), ("core",))
    specs = (PartitionSpec("core"),) * (n_in + 1)
    fn = jax.jit(
        shard_map(_body, mesh=mesh, in_specs=specs,
                  out_specs=(PartitionSpec("core"),), check_rep=False),
        donate_argnums=(n_in,), keep_unused=True)
    return fn, mesh, list(in_names)


def device_inputs(ins: dict, mesh, nreps: int = 1):
    """device_put the per-core host arrays (plus the nreps scalar)."""
    import jax
    from jax.sharding import NamedSharding, PartitionSpec

    sharding = NamedSharding(mesh, PartitionSpec("core"))
    d = {name: jax.device_put(np.concatenate(arrs, axis=0), sharding)
         for name, arrs in ins.items()}
    d["nreps"] = jax.device_put(np.full((N_CORES, 1), nreps, np.int32), sharding)
    return d, sharding


def kernel(x: np.ndarray, key_pts: np.ndarray, roll: np.ndarray, **_kw) -> np.ndarray:
    import jax

    x = np.ascontiguousarray(np.asarray(x, dtype=np.float32))
    key_pts = np.asarray(key_pts, dtype=np.float32)
    roll = np.asarray(roll, dtype=np.float32)

    w_items, ins, perm = build_schedule(key_pts, roll)
    nc = _get_module(w_items)
    fn, mesh, order = make_runner(nc)
    d, sharding = device_inputs(ins, mesh)
    out_g = jax.device_put(x[perm].reshape(nc_out_shape()), sharding)

    res = fn(*[d[n] for n in order], out_g)[0]
    res = np.asarray(res).reshape(N, C, H, W)
    final = np.empty_like(res)
    final[perm] = res
    return final


def nc_out_shape():
    return (N_CORES * (ROWS if MODE == "B" else ROWS6), HW if MODE == "B" else 512)
